# revision 28
# baseline (speedup 1.0000x reference)
"""ALiBi causal multi-head attention on 8 TRN2 NeuronCores.

Problem: x[2,2048,1024] -> qkv proj (16 heads, d=64) -> ALiBi-biased causal
softmax attention -> out proj [1024,1024] + bias.

Sharding: core = (batch b in {0,1}) x (head-group g in {0..3}, 4 heads each).
Head groups mix slope ranks -- GROUPS[g] is slot-ordered heaviest..lightest
(slot 0 = smallest slope = longest attention range). Each core computes its
batch's QKV for its 4 heads, causal attention, and a partial output
projection; host sums the 4 head-group partials per batch and adds b_out.

Key optimizations over the naive schedule:
  - ALiBi skipping: for a head with slope s, keys farther than ~W=8/s
    from the query contribute exp(<-8) ~ 0. Per-slot distance windows
    WSLOT=[1536,512,192,64] trim both which key tiles are computed AND the
    valid query-column range within each tile (numpy-verified: truncation
    adds 1.6e-4 in fp32; total ~7e-4 vs 2e-2 budget). This cuts sim+AV+exp
    by ~60% vs full causal. AV psum chains use split-start matmuls so the
    first writer of every psum column carries start=True (has_written bits
    are only cleared by start on the addressed range).
  - sim computed transposed (simT [keys, queries]); ALiBi bias folded into
    the sim matmul via two extra contraction rows (slope, -slope*i | j, 1);
    qT/kT zero-padded to 96 partitions (K<=64 matmuls run at half rate).
  - softmax denominator from a ones column appended to V (row 64 of psav).
  - Inputs ship host-pre-tiled to [128, .] layouts so each logical tensor
    is ONE contiguous DMA (17 input DMAs vs 67): per-DMA cost on a HWDGE
    ring is ~0.6us fixed + bytes/BW, strictly serialized per ring.
  - Input DMAs split across BOTH HWDGE rings (nc.sync + nc.scalar) in
    priority order (chunk-0 deps first); output DMAs ride the sync ring
    which is idle after ~18us. Attention starts ~10us (was ~45us).
  - PE emission interleaves sim tiles with independent GEMM work at
    2-matmul granularity (v/qk-proj of chunk c+1, out-proj of chunk c-1)
    so the PE never stalls waiting for Scalar exp and exp stays fed.
  - out-proj psum->sbuf copies are split between Scalar and Vector; in the
    last chunk Scalar is kept exp-only (copies go to Vector).
  - ~10 warmup matmuls bridge the HAM cold-clock window (PE runs at 1.2GHz
    until ~3.4us of sustained busy) while the first DMAs land.

HW lessons (NaN on hardware, fine in CoreSim -- do NOT reintroduce):
  - reciprocal_approx_fast reading PSUM directly produced NaN on HW
    (bisected: 3-D strided CAST out of PSUM is fine; cross-bank
    [128,1024] ACT reads and stride-0-broadcast tensor_add were removed
    together with it and remain untested in isolation).
  - BIR verifier requires engine-op partition offsets in {0,32,64,96}.
  - Scalar AF.Reciprocal is banned by bass (accuracy); use vector.
"""

import sys

for _p in ("/opt/trn_rl_repo", "/root/.axon_site/_ro/trn_rl_repo"):
    if _p not in sys.path:
        sys.path.append(_p)

import numpy as np
from math import log2, floor

import concourse.bass as bass
import concourse.mybir as mybir
import concourse.tile as tile
from concourse import bacc, bass_utils

F32 = mybir.dt.float32
F16 = mybir.dt.float16
AF = mybir.ActivationFunctionType

B = 2          # batches
NH = 16        # total heads
H = 4          # heads (slots) per core
D = 64         # head dim
N = 2048       # sequence length
DM = 1024      # model dim
CH = 512       # query chunk
NCH = N // CH  # 4
KD = DM // 128 # 8 contraction tiles for projections
SCALE = D ** -0.5
MASK_NEG = -30000.0
N_WARMUP = 9

# slot-ordered head groups (heaviest slope-rank first)
GROUPS = [[15, 11, 7, 3], [14, 10, 6, 2], [13, 9, 5, 1], [12, 8, 4, 0]]
# per-slot ALiBi windows: off-diag tiles as (k_back, hi_cols) nearest
# first, and per-diag-tile hi column limits (numpy-verified, err 1.6e-4)
OFFS = {
    0: [(1, 512), (2, 512), (3, 512), (4, 512), (5, 512), (6, 512),
        (7, 512), (8, 512), (9, 512), (10, 384)],
    1: [(1, 512), (2, 384), (3, 256)],
    2: [(1, 192)],
    3: [(1, 64)],
}
DIAG_HI = {
    0: [512, 512, 512, 512],
    1: [512, 512, 512, 512],
    2: [320, 448, 512, 512],
    3: [192, 320, 448, 512],
}


def _slopes(heads):
    def pow2_slopes(n):
        start = 2 ** (-(2 ** (-(log2(n) - 3))))
        return [start * (start ** i) for i in range(n)]
    if log2(heads).is_integer():
        return pow2_slopes(heads)
    c = 2 ** floor(log2(heads))
    return pow2_slopes(c) + pow2_slopes(2 * c)[0::2][: heads - c]


def build_program():
    nc = bacc.Bacc("TRN2", target_bir_lowering=False, debug=False, num_devices=8)
    # host-pre-tiled inputs: each is one contiguous [128, .] DMA
    xtm = nc.dram_tensor("xtm", [128, NCH, KD * CH], F16, kind="ExternalInput").ap()
    wq = nc.dram_tensor("wq", [128, KD * H * D], F16, kind="ExternalInput").ap()
    wk = nc.dram_tensor("wk", [128, KD * H * D], F16, kind="ExternalInput").ap()
    wv = nc.dram_tensor("wv", [128, KD * H * D], F16, kind="ExternalInput").ap()
    wo = nc.dram_tensor("wo", [128, 2 * DM], F16, kind="ExternalInput").ap()
    qaug = nc.dram_tensor("qaug", [H, 32, N], F16, kind="ExternalInput").ap()
    kaug = nc.dram_tensor("kaug", [32, N], F16, kind="ExternalInput").ap()
    m0 = nc.dram_tensor("m0", [128, 128], F32, kind="ExternalInput").ap()
    out = nc.dram_tensor("out", [N, DM], F16, kind="ExternalOutput").ap()

    with tile.TileContext(nc) as tc:
        with tc.tile_pool(name="persist", bufs=1) as cp:
            # ---- sync-ring input DMAs, priority order (chunk-0 deps first)
            xtc = []
            for c in range(NCH):
                t = cp.tile([128, KD * CH], F16, tag=f"xtc{c}", name=f"xtc{c}")
                xtc.append(t)
            wv_sb = cp.tile([128, KD * H * D], F16, tag="wv", name="wv_sb")
            wq_sb = cp.tile([128, KD * H * D], F16, tag="wq", name="wq_sb")
            wk_sb = cp.tile([128, KD * H * D], F16, tag="wk", name="wk_sb")
            wo_sb = cp.tile([128, 2 * DM], F16, tag="wo", name="wo_sb")

            # qk-proj gates the first sims: wq/xtc0-half on sync ring in
            # parallel with m0/wk/xtc0-half on the scalar ring, then aug.
            QKC = KD * CH // 4
            nc.sync.dma_start(wq_sb[:], wq[:])
            nc.sync.dma_start(xtc[0][:, 0:QKC], xtm[:, 0, 0:QKC])
            nc.sync.dma_start(xtc[0][:, QKC:2 * QKC], xtm[:, 0, QKC:2 * QKC])
            nc.sync.dma_start(wv_sb[:], wv[:])
            nc.sync.dma_start(xtc[1][:], xtm[:, 1, :])
            nc.sync.dma_start(xtc[3][:], xtm[:, 3, :])

            m0_sb = cp.tile([128, 128], F32, tag="m0", name="m0_sb")
            nc.scalar.dma_start(m0_sb[:], m0[:])
            nc.scalar.dma_start(wk_sb[:], wk[:])
            nc.scalar.dma_start(xtc[0][:, 2 * QKC:3 * QKC],
                                xtm[:, 0, 2 * QKC:3 * QKC])
            nc.scalar.dma_start(xtc[0][:, 3 * QKC:4 * QKC],
                                xtm[:, 0, 3 * QKC:4 * QKC])
            qt, kt = [], []
            for s in range(H):
                tk = cp.tile([96, N], F16, tag=f"kt{s}", name=f"kt{s}")
                nc.scalar.dma_start(tk[64:96, :], kaug[:])
                kt.append(tk)
                tq = cp.tile([96, N], F16, tag=f"qt{s}", name=f"qt{s}")
                nc.scalar.dma_start(tq[64:96, :], qaug[s])
                qt.append(tq)
            nc.scalar.dma_start(xtc[2][:], xtm[:, 2, :])
            nc.scalar.dma_start(wo_sb[:], wo[:])

            vsb = []
            for r in range(N // 128):
                t = cp.tile([128, 65 * H], F16, tag=f"v{r}", name=f"v{r}")
                ones = t[:, 0:65 * H].rearrange(
                    "p (h f) -> p h f", f=65)[:, :, 64:65]
                nc.gpsimd.memset(ones, 1.0)
                vsb.append(t)

            avt = []
            for p in range(2):
                t = cp.tile([128, N], F16, tag=f"avt{p}", name=f"avt{p}")
                avt.append(t)

            warm = cp.tile([128, CH], F16, tag="warm", name="warm")
            nc.vector.memset(warm[:], 0.0)


            with tc.tile_pool(name="psgemm", bufs=3, space="PSUM") as psg, \
                 tc.tile_pool(name="pssim", bufs=3, space="PSUM") as pss, \
                 tc.tile_pool(name="psav", bufs=2, space="PSUM") as psa, \
                 tc.tile_pool(name="ptp", bufs=36) as ptp, \
                 tc.tile_pool(name="smsb", bufs=3) as smsb, \
                 tc.tile_pool(name="osb", bufs=3) as osb:

                # ---------- GEMM granule generators (filler work) ----------
                def v_chain_granules(c, r):
                    # v rows for key tile r (chunk c columns of x^T);
                    # 4 granules of 2 matmuls, copy rides the last
                    box = {}

                    def mk(j):
                        def run():
                            if j == 0:
                                box["ps"] = psg.tile([128, CH], F32, tag="g",
                                                     name=f"psv{r}")
                            ps = box["ps"]
                            for k in (2 * j, 2 * j + 1):
                                nc.tensor.matmul(
                                    ps[:, 0:H * D],
                                    xtc[c][:, CH * k + 128 * (r % 4):
                                           CH * k + 128 * (r % 4) + 128],
                                    wv_sb[:, H * D * k:H * D * (k + 1)],
                                    start=(k == 0), stop=(k == KD - 1))
                            if j == 3:
                                dst = vsb[r][:, 0:H * 65].rearrange(
                                    "p (h f) -> p h f", f=65)[:, :, 0:64]
                                src = ps[:, 0:H * D].rearrange(
                                    "p (h f) -> p h f", f=64)
                                nc.vector.tensor_copy(dst, src)
                        return run
                    for j in range(4):
                        yield mk(j)

                def qk_chain_granules(c, hp, which, copy_split):
                    wsb, dst = (wq_sb, qt) if which == 0 else (wk_sb, kt)
                    box = {}

                    def mk(j):
                        def run():
                            if j == 0:
                                box["ps"] = psg.tile(
                                    [128, CH], F32, tag="g",
                                    name=f"psqk{c}_{hp}_{which}")
                            ps = box["ps"]
                            for k in (2 * j, 2 * j + 1):
                                nc.tensor.matmul(
                                    ps[:],
                                    wsb[:, H * D * k + 128 * hp:
                                        H * D * k + 128 * (hp + 1)],
                                    xtc[c][:, CH * k:CH * (k + 1)],
                                    start=(k == 0), stop=(k == KD - 1))
                            if j == 3:
                                eng0 = nc.scalar if copy_split else nc.vector
                                if copy_split:
                                    nc.scalar.copy(
                                        dst[2 * hp][0:64, CH * c:CH * (c + 1)],
                                        ps[0:64, :])
                                else:
                                    nc.vector.tensor_copy(
                                        dst[2 * hp][0:64, CH * c:CH * (c + 1)],
                                        ps[0:64, :])
                                nc.vector.tensor_copy(
                                    dst[2 * hp + 1][0:64, CH * c:CH * (c + 1)],
                                    ps[64:128, :])
                        return run
                    for j in range(4):
                        yield mk(j)

                def out_chain_granules(u, nchk, copy_eng):
                    # one granule: 2 matmuls + copy + half-tile DMA (halves
                    # alternate rings so the final drain is 2-wide)
                    def run():
                        ps = psg.tile([128, CH], F32, tag="g",
                                      name=f"pso{u}_{nchk}")
                        for kk in range(2):
                            nc.tensor.matmul(
                                ps[:],
                                avt[kk][:, 128 * u:128 * (u + 1)],
                                wo_sb[:, DM * kk + CH * nchk:
                                      DM * kk + CH * (nchk + 1)],
                                start=(kk == 0), stop=(kk == 1))
                        o_sb = osb_tiles[u]
                        if copy_eng == "s":
                            nc.scalar.activation(
                                o_sb[:, CH * nchk:CH * (nchk + 1)], ps[:],
                                AF.Copy)
                        else:
                            nc.vector.tensor_copy(
                                o_sb[:, CH * nchk:CH * (nchk + 1)], ps[:])
                        ring = nc.sync if (u + nchk) % 2 == 0 else nc.scalar
                        ring.dma_start(
                            out[128 * u:128 * (u + 1),
                                CH * nchk:CH * (nchk + 1)],
                            o_sb[:, CH * nchk:CH * (nchk + 1)])
                    yield run

                osb_tiles = {}

                def fill_units(c):
                    """Filler granules to interleave into chunk c's
                    attention: v-proj(c, then c+1), qk-proj(c+1),
                    out-proj(c-1). In the last chunk Scalar stays
                    exp-only."""
                    last = (c == NCH - 1)
                    if c == 0:
                        for w in range(2):
                            yield from qk_chain_granules(0, 1, w, False)
                        for r in range(4):
                            yield from v_chain_granules(0, r)
                    if c + 1 < NCH:
                        for r in range(4 * (c + 1), 4 * (c + 1) + 4):
                            yield from v_chain_granules(c + 1, r)
                        for hp in range(H // 2):
                            for w in range(2):
                                # qk copies for the c3 projections (emitted
                                # during c2) split between Scalar/Vector
                                yield from qk_chain_granules(
                                    c + 1, hp, w, copy_split=(c == 2))
                    j = c - 1
                    if j >= 0:
                        for u in range(4 * j, 4 * j + 4):
                            osb_tiles[u] = osb.tile([128, DM], F16, tag="osb",
                                                    name=f"osb{u}")
                            for nchk in range(2):
                                eng = "v" if last else ("s" if nchk else "v")
                                yield from out_chain_granules(u, nchk, eng)

                # ---------- attention emitters ----------
                def emit_sim(c, s, t, lo, hi, diag):
                    """sim for (slot s, key tile t) over chunk-c cols
                    lo..hi; diag tiles get the causal mask add.
                    NOTE: accumulating the mask on the PE (ident @ m0 into
                    the open psum group with a partial-column stop) NaN'd
                    on HW while passing CoreSim -- do not reintroduce."""
                    ps = pss.tile([128, CH], F32, tag="sim",
                                  name=f"sim{c}_{t}_{s}")
                    nc.tensor.matmul(
                        ps[:, lo:hi],
                        kt[s][0:96, 128 * t:128 * (t + 1)],
                        qt[s][0:96, CH * c + lo:CH * c + hi],
                        start=True, stop=True)
                    if diag:
                        # causal mask add (Vector; GpSimd cannot access PSUM
                        # per the BIR verifier, Scalar has no tensor_tensor)
                        nc.vector.tensor_add(
                            ps[:, lo:lo + 128], ps[:, lo:lo + 128], m0_sb[:])
                    return ps

                def emit_exp(ps, c, s, t, lo, hi):
                    pt = ptp.tile([128, CH], F16, tag="pt",
                                  name=f"pt{c}_{t}_{s}")
                    nc.scalar.activation(pt[:, lo:hi], ps[:, lo:hi], AF.Exp)
                    return pt

                # ---------- main pipeline ----------
                ps_w = psg.tile([128, CH], F32, tag="g", name="ps_warm")
                for i in range(N_WARMUP):
                    nc.tensor.matmul(ps_w[:], warm[:, 0:128], warm[:],
                                     start=True, stop=True)
                # prologue: only the hp0 q/k chains gate the first sims
                # (slots 0/1); hp1 chains ride the chunk-0 fill
                for w in range(2):
                    for g in qk_chain_granules(0, 0, w, False):
                        g()

                for c in range(NCH):
                    fill = fill_units(c)
                    done_fill = False

                    def take_fill(k=1):
                        nonlocal done_fill
                        for _ in range(k):
                            if done_fill:
                                return
                            u = next(fill, None)
                            if u is None:
                                done_fill = True
                                return
                            u()

                    # chunk's sim jobs: per slot windowed off-diag tiles
                    # (farthest first), then 4 diag tiles (masked).
                    av_tiles = {s: [] for s in range(H)}  # s -> [(pt,lo,hi,t)]
                    sim_jobs = []
                    for s in range(H):
                        for (k, hi) in OFFS[s]:  # nearest first
                            t = 4 * c - k
                            if t >= 0:
                                sim_jobs.append((s, t, 0, hi, False))
                    for s in range(H):
                        for d in range(4):
                            sim_jobs.append(
                                (s, 4 * c + d, 128 * d, DIAG_HI[s][d], True))

                    # spread filler granules evenly across the sim jobs
                    n_gran = {0: 56, 1: 40, 2: 40, 3: 8}[c]
                    ns = len(sim_jobs)
                    for i, (s, t, lo, hi, diag) in enumerate(sim_jobs):
                        ps = emit_sim(c, s, t, lo, hi, diag)
                        take_fill((i + 1) * n_gran // ns - i * n_gran // ns)
                        pt = emit_exp(ps, c, s, t, lo, hi)
                        av_tiles[s].append((pt, lo, hi, t))
                    # drain remaining filler before AV so exp gets ahead
                    while not done_fill:
                        take_fill(1)

                    # AV + normalize per slot. The psum accumulation group
                    # must be opened by ONE start=True matmul covering all
                    # columns later written: slots 0/1 start with a
                    # full-width tile; windowed slots 2/3 open with a
                    # full-width zero matmul (rhs = zero `warm` tile).
                    for s in range(H):
                        ps_av = psa.tile([65, CH], F32, tag="av",
                                         name=f"psav{s}_{c}")
                        tiles = av_tiles[s]
                        full_first = tiles[0][1] == 0 and tiles[0][2] == CH
                        if not full_first:
                            nc.tensor.matmul(
                                ps_av[:], vsb[4 * c][:, 65 * s:65 * s + 65],
                                warm[:], start=True, stop=False)
                        for idx, (pt, lo, hi, t) in enumerate(tiles):
                            nc.tensor.matmul(
                                ps_av[:, lo:hi],
                                vsb[t][:, 65 * s:65 * s + 65],
                                pt[:, lo:hi],
                                start=(full_first and idx == 0),
                                stop=(idx == len(tiles) - 1))
                        # NOTE: batching slot-pair denominators at partition
                        # offsets {0,32} with one [33,CH] reciprocal + a
                        # partition_broadcast from offset 32 gave WRONG
                        # results on HW (CoreSim clean) -- keep per-slot.
                        dn32 = smsb.tile([1, CH], F32, tag="dn",
                                         name=f"dn{s}_{c}")
                        if c == NCH - 1:
                            nc.vector.tensor_copy(dn32[:], ps_av[64:65, :])
                        else:
                            nc.scalar.activation(dn32[:], ps_av[64:65, :],
                                                 AF.Copy)
                        rc32 = smsb.tile([1, CH], F32, tag="rc",
                                         name=f"rc{s}_{c}")
                        nc.vector.reciprocal_approx_fast(rc32[:], dn32[:])
                        rcb = smsb.tile([D, CH], F32, tag="rcb",
                                        name=f"rcb{s}_{c}")
                        nc.gpsimd.partition_broadcast(rcb[:], rc32[:])
                        nc.vector.tensor_mul(
                            avt[s // 2][64 * (s % 2):64 * (s % 2) + 64,
                                        CH * c:CH * (c + 1)],
                            ps_av[0:64, :], rcb[:])

                # epilogue: out-proj for the last chunk (Scalar is free:
                # split copies between Scalar and Vector)
                for u in range(4 * (NCH - 1), 4 * (NCH - 1) + 4):
                    osb_tiles[u] = osb.tile([128, DM], F16, tag="osb",
                                            name=f"osb{u}")
                    for nchk in range(2):
                        for g in out_chain_granules(
                                u, nchk, "s" if nchk == 0 else "v"):
                            g()

    nc.compile()
    return nc


def make_in_maps(x, w_qkv, w_out):
    """Per-core numpy input dicts. Core c = batch (c // 4) x head-group
    GROUPS[c % 4] (slot-ordered). All matrices host-pre-tiled to [128, .]
    so each is one contiguous DMA."""
    slopes = _slopes(NH)
    pos = np.arange(N, dtype=np.float32)
    kaug = np.zeros((32, N), np.float16)
    kaug[0] = pos.astype(np.float16)
    kaug[1] = 1.0
    m0 = np.where(np.arange(128)[:, None] > np.arange(128)[None, :],
                  np.float32(MASK_NEG), np.float32(0.0))

    def tile128(w):  # [KD*128, F] -> [128, KD*F] (k-major free layout)
        kd, f = w.shape[0] // 128, w.shape[1]
        return np.ascontiguousarray(
            w.reshape(kd, 128, f).transpose(1, 0, 2).reshape(128, kd * f))

    # x^T pre-tiled: [128, NCH, KD*CH]; chunk c holds k-major 512-col blocks
    xtm = []
    for b in range(B):
        xT = np.ascontiguousarray(x[b].T).astype(np.float16)  # [DM, N]
        t = xT.reshape(KD, 128, NCH, CH).transpose(1, 2, 0, 3)
        xtm.append(np.ascontiguousarray(t.reshape(128, NCH, KD * CH)))

    in_maps = []
    for cidx in range(8):
        b, g = cidx // 4, cidx % 4
        heads = GROUPS[g]
        wq = np.concatenate(
            [(w_qkv[:, h * D:(h + 1) * D] * SCALE) for h in heads],
            1).astype(np.float16)
        wk = np.concatenate(
            [w_qkv[:, DM + h * D:DM + (h + 1) * D] for h in heads],
            1).astype(np.float16)
        wv = np.concatenate(
            [w_qkv[:, 2 * DM + h * D:2 * DM + (h + 1) * D] for h in heads],
            1).astype(np.float16)
        wo = np.concatenate(
            [w_out[h * D:(h + 1) * D, :] for h in heads], 0).astype(np.float16)
        qa = np.zeros((H, 32, N), np.float16)
        for s, h in enumerate(heads):
            s16 = np.float16(slopes[h])
            qa[s, 0, :] = s16
            qa[s, 1, :] = (-np.float32(s16) * pos).astype(np.float16)
        in_maps.append({
            "xtm": xtm[b], "wq": tile128(wq), "wk": tile128(wk),
            "wv": tile128(wv), "wo": tile128(wo),
            "qaug": qa, "kaug": kaug, "m0": m0,
        })
    return in_maps


_NC_CACHE = []


def _get_nc():
    if not _NC_CACHE:
        _NC_CACHE.append(build_program())
    return _NC_CACHE[0]


def run_cores(in_maps, **kw):
    nc = _get_nc()
    return bass_utils.run_bass_kernel_spmd(nc, in_maps, core_ids=list(range(8)), **kw)


def kernel(x, w_qkv, w_out, b_out):
    x = np.asarray(x, np.float32)
    w_qkv = np.asarray(w_qkv, np.float32)
    w_out = np.asarray(w_out, np.float32)
    b_out = np.asarray(b_out, np.float32)
    res = run_cores(make_in_maps(x, w_qkv, w_out))
    out = np.zeros((B, N, DM), np.float32)
    for c in range(8):
        out[c // 4] += res.results[c]["out"].astype(np.float32)
    out += b_out[None, None, :]
    return out


# revision 31
# speedup vs baseline: 1.0253x; 1.0253x over previous
"""ALiBi causal multi-head attention on 8 TRN2 NeuronCores.

Problem: x[2,2048,1024] -> qkv proj (16 heads, d=64) -> ALiBi-biased causal
softmax attention -> out proj [1024,1024] + bias.

Sharding: core = (batch b in {0,1}) x (head-group g in {0..3}, 4 heads each).
Head groups mix slope ranks -- GROUPS[g] is slot-ordered heaviest..lightest
(slot 0 = smallest slope = longest attention range). Each core computes its
batch's QKV for its 4 heads, causal attention, and a partial output
projection; host sums the 4 head-group partials per batch and adds b_out.

Key optimizations over the naive schedule:
  - ALiBi skipping: for a head with slope s, keys farther than ~W=8/s
    from the query contribute exp(<-8) ~ 0. Per-slot distance windows
    WSLOT=[1536,512,192,64] trim both which key tiles are computed AND the
    valid query-column range within each tile (numpy-verified: truncation
    adds 1.6e-4 in fp32; total ~7e-4 vs 2e-2 budget). This cuts sim+AV+exp
    by ~60% vs full causal. AV psum chains use split-start matmuls so the
    first writer of every psum column carries start=True (has_written bits
    are only cleared by start on the addressed range).
  - sim computed transposed (simT [keys, queries]); ALiBi bias folded into
    the sim matmul via two extra contraction rows (slope, -slope*i | j, 1);
    qT/kT zero-padded to 96 partitions (K<=64 matmuls run at half rate).
  - softmax denominator from a ones column appended to V (row 64 of psav).
  - Inputs ship host-pre-tiled to [128, .] layouts so each logical tensor
    is ONE contiguous DMA (17 input DMAs vs 67): per-DMA cost on a HWDGE
    ring is ~0.6us fixed + bytes/BW, strictly serialized per ring.
  - Input DMAs split across BOTH HWDGE rings (nc.sync + nc.scalar) in
    priority order (chunk-0 deps first); output DMAs ride the sync ring
    which is idle after ~18us. Attention starts ~10us (was ~45us).
  - PE emission interleaves sim tiles with independent GEMM work at
    2-matmul granularity (v/qk-proj of chunk c+1, out-proj of chunk c-1)
    so the PE never stalls waiting for Scalar exp and exp stays fed.
  - out-proj psum->sbuf copies are split between Scalar and Vector; in the
    last chunk Scalar is kept exp-only (copies go to Vector).
  - ~10 warmup matmuls bridge the HAM cold-clock window (PE runs at 1.2GHz
    until ~3.4us of sustained busy) while the first DMAs land.

HW lessons (NaN on hardware, fine in CoreSim -- do NOT reintroduce):
  - reciprocal_approx_fast reading PSUM directly produced NaN on HW
    (bisected: 3-D strided CAST out of PSUM is fine; cross-bank
    [128,1024] ACT reads and stride-0-broadcast tensor_add were removed
    together with it and remain untested in isolation).
  - BIR verifier requires engine-op partition offsets in {0,32,64,96}.
  - Scalar AF.Reciprocal is banned by bass (accuracy); use vector.
"""

import sys

for _p in ("/opt/trn_rl_repo", "/root/.axon_site/_ro/trn_rl_repo"):
    if _p not in sys.path:
        sys.path.append(_p)

import numpy as np
from math import log2, floor

import concourse.bass as bass
import concourse.mybir as mybir
import concourse.tile as tile
from concourse import bacc, bass_utils

F32 = mybir.dt.float32
F16 = mybir.dt.float16
AF = mybir.ActivationFunctionType

B = 2          # batches
NH = 16        # total heads
H = 4          # heads (slots) per core
D = 64         # head dim
N = 2048       # sequence length
DM = 1024      # model dim
CH = 512       # query chunk
NCH = N // CH  # 4
KD = DM // 128 # 8 contraction tiles for projections
SCALE = D ** -0.5
MASK_NEG = -30000.0
N_WARMUP = 9

# slot-ordered head groups (heaviest slope-rank first)
GROUPS = [[15, 11, 7, 3], [14, 10, 6, 2], [13, 9, 5, 1], [12, 8, 4, 0]]
# per-slot ALiBi windows: off-diag tiles as (k_back, hi_cols) nearest
# first, and per-diag-tile hi column limits (numpy-verified, err 1.6e-4)
OFFS = {
    0: [(1, 512), (2, 512), (3, 512), (4, 512), (5, 512), (6, 512),
        (7, 512), (8, 512), (9, 512), (10, 384)],
    1: [(1, 512), (2, 384), (3, 256)],
    2: [(1, 192)],
    3: [(1, 64)],
}
DIAG_HI = {
    0: [512, 512, 512, 512],
    1: [512, 512, 512, 512],
    2: [320, 448, 512, 512],
    3: [192, 320, 448, 512],
}


def _slopes(heads):
    def pow2_slopes(n):
        start = 2 ** (-(2 ** (-(log2(n) - 3))))
        return [start * (start ** i) for i in range(n)]
    if log2(heads).is_integer():
        return pow2_slopes(heads)
    c = 2 ** floor(log2(heads))
    return pow2_slopes(c) + pow2_slopes(2 * c)[0::2][: heads - c]


def build_program():
    nc = bacc.Bacc("TRN2", target_bir_lowering=False, debug=False, num_devices=8)
    # host-pre-tiled inputs: each is one contiguous [128, .] DMA
    xtm = nc.dram_tensor("xtm", [128, NCH, KD * CH], F16, kind="ExternalInput").ap()
    wq = nc.dram_tensor("wq", [128, KD * H * D], F16, kind="ExternalInput").ap()
    wk = nc.dram_tensor("wk", [128, KD * H * D], F16, kind="ExternalInput").ap()
    wv = nc.dram_tensor("wv", [128, KD * H * D], F16, kind="ExternalInput").ap()
    wo = nc.dram_tensor("wo", [128, 2 * DM], F16, kind="ExternalInput").ap()
    qaug = nc.dram_tensor("qaug", [H, 32, N], F16, kind="ExternalInput").ap()
    kaug = nc.dram_tensor("kaug", [32, N], F16, kind="ExternalInput").ap()
    m0 = nc.dram_tensor("m0", [128, 128], F32, kind="ExternalInput").ap()
    out = nc.dram_tensor("out", [N, DM], F16, kind="ExternalOutput").ap()

    with tile.TileContext(nc) as tc:
        with tc.tile_pool(name="persist", bufs=1) as cp:
            # ---- sync-ring input DMAs, priority order (chunk-0 deps first)
            xtc = []
            for c in range(NCH):
                t = cp.tile([128, KD * CH], F16, tag=f"xtc{c}", name=f"xtc{c}")
                xtc.append(t)
            wv_sb = cp.tile([128, KD * H * D], F16, tag="wv", name="wv_sb")
            wq_sb = cp.tile([128, KD * H * D], F16, tag="wq", name="wq_sb")
            wk_sb = cp.tile([128, KD * H * D], F16, tag="wk", name="wk_sb")
            wo_sb = cp.tile([128, 2 * DM], F16, tag="wo", name="wo_sb")

            # qk-proj gates the first sims: wq/xtc0-half on sync ring in
            # parallel with m0/wk/xtc0-half on the scalar ring, then aug.
            QKC = KD * CH // 4
            nc.sync.dma_start(wq_sb[:], wq[:])
            nc.sync.dma_start(xtc[0][:, 0:QKC], xtm[:, 0, 0:QKC])
            nc.sync.dma_start(xtc[0][:, QKC:2 * QKC], xtm[:, 0, QKC:2 * QKC])
            nc.sync.dma_start(wv_sb[:], wv[:])
            nc.sync.dma_start(xtc[1][:], xtm[:, 1, :])
            nc.sync.dma_start(xtc[3][:], xtm[:, 3, :])

            m0_sb = cp.tile([128, 128], F32, tag="m0", name="m0_sb")
            nc.scalar.dma_start(m0_sb[:], m0[:])
            nc.scalar.dma_start(wk_sb[:], wk[:])
            nc.scalar.dma_start(xtc[0][:, 2 * QKC:3 * QKC],
                                xtm[:, 0, 2 * QKC:3 * QKC])
            nc.scalar.dma_start(xtc[0][:, 3 * QKC:4 * QKC],
                                xtm[:, 0, 3 * QKC:4 * QKC])
            qt, kt = [], []
            for s in range(H):
                tk = cp.tile([96, N], F16, tag=f"kt{s}", name=f"kt{s}")
                nc.scalar.dma_start(tk[64:96, :], kaug[:])
                kt.append(tk)
                tq = cp.tile([96, N], F16, tag=f"qt{s}", name=f"qt{s}")
                nc.scalar.dma_start(tq[64:96, :], qaug[s])
                qt.append(tq)
            nc.scalar.dma_start(xtc[2][:], xtm[:, 2, :])
            nc.scalar.dma_start(wo_sb[:], wo[:])

            vsb = []
            for r in range(N // 128):
                t = cp.tile([128, 65 * H], F16, tag=f"v{r}", name=f"v{r}")
                ones = t[:, 0:65 * H].rearrange(
                    "p (h f) -> p h f", f=65)[:, :, 64:65]
                nc.gpsimd.memset(ones, 1.0)
                vsb.append(t)

            avt = []
            for p in range(2):
                t = cp.tile([128, N], F16, tag=f"avt{p}", name=f"avt{p}")
                avt.append(t)

            warm = cp.tile([128, CH], F16, tag="warm", name="warm")
            nc.vector.memset(warm[:], 0.0)


            with tc.tile_pool(name="psgemm", bufs=3, space="PSUM") as psg, \
                 tc.tile_pool(name="pssim", bufs=3, space="PSUM") as pss, \
                 tc.tile_pool(name="psav", bufs=2, space="PSUM") as psa, \
                 tc.tile_pool(name="ptp", bufs=36) as ptp, \
                 tc.tile_pool(name="smsb", bufs=3) as smsb, \
                 tc.tile_pool(name="osb", bufs=3) as osb:

                # ---------- GEMM granule generators (filler work) ----------
                def v_chain_granules(c, r):
                    # v rows for key tile r (chunk c columns of x^T);
                    # 4 granules of 2 matmuls, copy rides the last
                    box = {}

                    def mk(j):
                        def run():
                            if j == 0:
                                box["ps"] = psg.tile([128, CH], F32, tag="g",
                                                     name=f"psv{r}")
                            ps = box["ps"]
                            for k in (2 * j, 2 * j + 1):
                                nc.tensor.matmul(
                                    ps[:, 0:H * D],
                                    xtc[c][:, CH * k + 128 * (r % 4):
                                           CH * k + 128 * (r % 4) + 128],
                                    wv_sb[:, H * D * k:H * D * (k + 1)],
                                    start=(k == 0), stop=(k == KD - 1))
                            if j == 3:
                                dst = vsb[r][:, 0:H * 65].rearrange(
                                    "p (h f) -> p h f", f=65)[:, :, 0:64]
                                src = ps[:, 0:H * D].rearrange(
                                    "p (h f) -> p h f", f=64)
                                nc.vector.tensor_copy(dst, src)
                        return run
                    for j in range(4):
                        yield mk(j)

                def qk_chain_granules(c, hp, which, copy_split):
                    wsb, dst = (wq_sb, qt) if which == 0 else (wk_sb, kt)
                    box = {}

                    def mk(j):
                        def run():
                            if j == 0:
                                box["ps"] = psg.tile(
                                    [128, CH], F32, tag="g",
                                    name=f"psqk{c}_{hp}_{which}")
                            ps = box["ps"]
                            for k in (2 * j, 2 * j + 1):
                                nc.tensor.matmul(
                                    ps[:],
                                    wsb[:, H * D * k + 128 * hp:
                                        H * D * k + 128 * (hp + 1)],
                                    xtc[c][:, CH * k:CH * (k + 1)],
                                    start=(k == 0), stop=(k == KD - 1))
                            if j == 3:
                                eng0 = nc.scalar if copy_split else nc.vector
                                if copy_split:
                                    nc.scalar.copy(
                                        dst[2 * hp][0:64, CH * c:CH * (c + 1)],
                                        ps[0:64, :])
                                else:
                                    nc.vector.tensor_copy(
                                        dst[2 * hp][0:64, CH * c:CH * (c + 1)],
                                        ps[0:64, :])
                                nc.vector.tensor_copy(
                                    dst[2 * hp + 1][0:64, CH * c:CH * (c + 1)],
                                    ps[64:128, :])
                        return run
                    for j in range(4):
                        yield mk(j)

                def out_chain_granules(u, nchk, copy_eng):
                    # one granule: 2 matmuls + copy + half-tile DMA (halves
                    # alternate rings so the final drain is 2-wide)
                    def run():
                        ps = psg.tile([128, CH], F32, tag="g",
                                      name=f"pso{u}_{nchk}")
                        for kk in range(2):
                            nc.tensor.matmul(
                                ps[:],
                                avt[kk][:, 128 * u:128 * (u + 1)],
                                wo_sb[:, DM * kk + CH * nchk:
                                      DM * kk + CH * (nchk + 1)],
                                start=(kk == 0), stop=(kk == 1))
                        o_sb = osb_tiles[u]
                        if copy_eng == "s":
                            nc.scalar.activation(
                                o_sb[:, CH * nchk:CH * (nchk + 1)], ps[:],
                                AF.Copy)
                        else:
                            nc.vector.tensor_copy(
                                o_sb[:, CH * nchk:CH * (nchk + 1)], ps[:])
                        ring = nc.sync if (u + nchk) % 2 == 0 else nc.scalar
                        ring.dma_start(
                            out[128 * u:128 * (u + 1),
                                CH * nchk:CH * (nchk + 1)],
                            o_sb[:, CH * nchk:CH * (nchk + 1)])
                    yield run

                osb_tiles = {}

                def fill_units(c):
                    """Filler granules to interleave into chunk c's
                    attention: v-proj(c, then c+1), qk-proj(c+1),
                    out-proj(c-1). In the last chunk Scalar stays
                    exp-only."""
                    last = (c == NCH - 1)
                    if c == 0:
                        for w in range(2):
                            yield from qk_chain_granules(0, 1, w, False)
                        for r in range(4):
                            yield from v_chain_granules(0, r)
                    if c + 1 < NCH:
                        for r in range(4 * (c + 1), 4 * (c + 1) + 4):
                            yield from v_chain_granules(c + 1, r)
                        for hp in range(H // 2):
                            for w in range(2):
                                # qk copies for the c3 projections (emitted
                                # during c2) split between Scalar/Vector
                                yield from qk_chain_granules(
                                    c + 1, hp, w, copy_split=(c == 2))
                    j = c - 1
                    if j >= 0:
                        for u in range(4 * j, 4 * j + 4):
                            osb_tiles[u] = osb.tile([128, DM], F16, tag="osb",
                                                    name=f"osb{u}")
                            for nchk in range(2):
                                eng = "v" if last else ("s" if nchk else "v")
                                yield from out_chain_granules(u, nchk, eng)

                # ---------- attention emitters ----------
                def emit_sim(c, s, t, lo, hi, diag):
                    """sim for (slot s, key tile t) over chunk-c cols
                    lo..hi; diag tiles get the causal mask add.
                    NOTE: accumulating the mask on the PE (ident @ m0 into
                    the open psum group with a partial-column stop) NaN'd
                    on HW while passing CoreSim -- do not reintroduce."""
                    ps = pss.tile([128, CH], F32, tag="sim",
                                  name=f"sim{c}_{t}_{s}")
                    nc.tensor.matmul(
                        ps[:, lo:hi],
                        kt[s][0:96, 128 * t:128 * (t + 1)],
                        qt[s][0:96, CH * c + lo:CH * c + hi],
                        start=True, stop=True)
                    if diag:
                        # causal mask add (Vector; GpSimd cannot access PSUM
                        # per the BIR verifier, Scalar has no tensor_tensor)
                        nc.vector.tensor_add(
                            ps[:, lo:lo + 128], ps[:, lo:lo + 128], m0_sb[:])
                    return ps

                def emit_exp(ps, c, s, t, lo, hi):
                    pt = ptp.tile([128, CH], F16, tag="pt",
                                  name=f"pt{c}_{t}_{s}")
                    nc.scalar.activation(pt[:, lo:hi], ps[:, lo:hi], AF.Exp)
                    return pt

                def emit_av_norm(c, s, tiles):
                    """AV chain + softmax normalize for one slot. The psum
                    accumulation group must be opened by ONE start=True
                    matmul covering all columns later written: slots 0/1
                    start with a full-width tile; windowed slots 2/3 open
                    with a full-width zero matmul (rhs = zero `warm`)."""
                    ps_av = psa.tile([65, CH], F32, tag="av",
                                     name=f"psav{s}_{c}")
                    full_first = tiles[0][1] == 0 and tiles[0][2] == CH
                    if not full_first:
                        nc.tensor.matmul(
                            ps_av[:], vsb[4 * c][:, 65 * s:65 * s + 65],
                            warm[:], start=True, stop=False)
                    for idx, (pt, lo, hi, t) in enumerate(tiles):
                        nc.tensor.matmul(
                            ps_av[:, lo:hi],
                            vsb[t][:, 65 * s:65 * s + 65],
                            pt[:, lo:hi],
                            start=(full_first and idx == 0),
                            stop=(idx == len(tiles) - 1))
                    # NOTE: batching slot-pair denominators at partition
                    # offsets {0,32} with one [33,CH] reciprocal + a
                    # partition_broadcast from offset 32 gave WRONG
                    # results on HW (CoreSim clean) -- keep per-slot.
                    dn32 = smsb.tile([1, CH], F32, tag="dn",
                                     name=f"dn{s}_{c}")
                    if c == NCH - 1:
                        nc.vector.tensor_copy(dn32[:], ps_av[64:65, :])
                    else:
                        nc.scalar.activation(dn32[:], ps_av[64:65, :],
                                             AF.Copy)
                    rc32 = smsb.tile([1, CH], F32, tag="rc",
                                     name=f"rc{s}_{c}")
                    nc.vector.reciprocal_approx_fast(rc32[:], dn32[:])
                    rcb = smsb.tile([D, CH], F32, tag="rcb",
                                    name=f"rcb{s}_{c}")
                    nc.gpsimd.partition_broadcast(rcb[:], rc32[:])
                    nc.vector.tensor_mul(
                        avt[s // 2][64 * (s % 2):64 * (s % 2) + 64,
                                    CH * c:CH * (c + 1)],
                        ps_av[0:64, :], rcb[:])

                # ---------- main pipeline ----------
                ps_w = psg.tile([128, CH], F32, tag="g", name="ps_warm")
                for i in range(N_WARMUP):
                    nc.tensor.matmul(ps_w[:], warm[:, 0:128], warm[:],
                                     start=True, stop=True)
                # prologue: only the hp0 q/k chains gate the first sims
                # (slots 0/1); hp1 chains ride the chunk-0 fill
                for w in range(2):
                    for g in qk_chain_granules(0, 0, w, False):
                        g()

                for c in range(NCH):
                    fill = fill_units(c)
                    done_fill = False

                    def take_fill(k=1):
                        nonlocal done_fill
                        for _ in range(k):
                            if done_fill:
                                return
                            u = next(fill, None)
                            if u is None:
                                done_fill = True
                                return
                            u()

                    # chunk's sim jobs: all slots' windowed off-diag tiles
                    # first, then per slot 4 diag tiles; AV+norm of slot s
                    # is emitted after slot s+1's diag sims (one-slot lag
                    # keeps Scalar's exp queue ahead of the AV chain).
                    av_tiles = {s: [] for s in range(H)}  # s -> [(pt,lo,hi,t)]
                    sim_jobs = []
                    for s in range(H):
                        for (k, hi) in OFFS[s]:  # nearest first
                            t = 4 * c - k
                            if t >= 0:
                                sim_jobs.append((s, t, 0, hi, False))
                    for s in range(H):
                        for d in range(4):
                            sim_jobs.append(
                                (s, 4 * c + d, 128 * d, DIAG_HI[s][d], True))

                    # spread filler granules evenly across the sim jobs
                    n_gran = {0: 56, 1: 40, 2: 40, 3: 8}[c]
                    ns = len(sim_jobs)
                    for i, (s, t, lo, hi, diag) in enumerate(sim_jobs):
                        ps = emit_sim(c, s, t, lo, hi, diag)
                        take_fill((i + 1) * n_gran // ns - i * n_gran // ns)
                        pt = emit_exp(ps, c, s, t, lo, hi)
                        av_tiles[s].append((pt, lo, hi, t))
                        if diag and lo == 384 and s >= 1:
                            emit_av_norm(c, s - 1, av_tiles[s - 1])
                    # drain remaining filler before the last slot's AV
                    while not done_fill:
                        take_fill(1)
                    emit_av_norm(c, H - 1, av_tiles[H - 1])

                # epilogue: out-proj for the last chunk (Scalar is free:
                # split copies between Scalar and Vector)
                for u in range(4 * (NCH - 1), 4 * (NCH - 1) + 4):
                    osb_tiles[u] = osb.tile([128, DM], F16, tag="osb",
                                            name=f"osb{u}")
                    for nchk in range(2):
                        for g in out_chain_granules(
                                u, nchk, "s" if nchk == 0 else "v"):
                            g()

    nc.compile()
    return nc


def make_in_maps(x, w_qkv, w_out):
    """Per-core numpy input dicts. Core c = batch (c // 4) x head-group
    GROUPS[c % 4] (slot-ordered). All matrices host-pre-tiled to [128, .]
    so each is one contiguous DMA."""
    slopes = _slopes(NH)
    pos = np.arange(N, dtype=np.float32)
    kaug = np.zeros((32, N), np.float16)
    kaug[0] = pos.astype(np.float16)
    kaug[1] = 1.0
    m0 = np.where(np.arange(128)[:, None] > np.arange(128)[None, :],
                  np.float32(MASK_NEG), np.float32(0.0))

    def tile128(w):  # [KD*128, F] -> [128, KD*F] (k-major free layout)
        kd, f = w.shape[0] // 128, w.shape[1]
        return np.ascontiguousarray(
            w.reshape(kd, 128, f).transpose(1, 0, 2).reshape(128, kd * f))

    # x^T pre-tiled: [128, NCH, KD*CH]; chunk c holds k-major 512-col blocks
    xtm = []
    for b in range(B):
        xT = np.ascontiguousarray(x[b].T).astype(np.float16)  # [DM, N]
        t = xT.reshape(KD, 128, NCH, CH).transpose(1, 2, 0, 3)
        xtm.append(np.ascontiguousarray(t.reshape(128, NCH, KD * CH)))

    in_maps = []
    for cidx in range(8):
        b, g = cidx // 4, cidx % 4
        heads = GROUPS[g]
        wq = np.concatenate(
            [(w_qkv[:, h * D:(h + 1) * D] * SCALE) for h in heads],
            1).astype(np.float16)
        wk = np.concatenate(
            [w_qkv[:, DM + h * D:DM + (h + 1) * D] for h in heads],
            1).astype(np.float16)
        wv = np.concatenate(
            [w_qkv[:, 2 * DM + h * D:2 * DM + (h + 1) * D] for h in heads],
            1).astype(np.float16)
        wo = np.concatenate(
            [w_out[h * D:(h + 1) * D, :] for h in heads], 0).astype(np.float16)
        qa = np.zeros((H, 32, N), np.float16)
        for s, h in enumerate(heads):
            s16 = np.float16(slopes[h])
            qa[s, 0, :] = s16
            qa[s, 1, :] = (-np.float32(s16) * pos).astype(np.float16)
        in_maps.append({
            "xtm": xtm[b], "wq": tile128(wq), "wk": tile128(wk),
            "wv": tile128(wv), "wo": tile128(wo),
            "qaug": qa, "kaug": kaug, "m0": m0,
        })
    return in_maps


_NC_CACHE = []


def _get_nc():
    if not _NC_CACHE:
        _NC_CACHE.append(build_program())
    return _NC_CACHE[0]


def run_cores(in_maps, **kw):
    nc = _get_nc()
    return bass_utils.run_bass_kernel_spmd(nc, in_maps, core_ids=list(range(8)), **kw)


def kernel(x, w_qkv, w_out, b_out):
    x = np.asarray(x, np.float32)
    w_qkv = np.asarray(w_qkv, np.float32)
    w_out = np.asarray(w_out, np.float32)
    b_out = np.asarray(b_out, np.float32)
    res = run_cores(make_in_maps(x, w_qkv, w_out))
    out = np.zeros((B, N, DM), np.float32)
    for c in range(8):
        out[c // 4] += res.results[c]["out"].astype(np.float32)
    out += b_out[None, None, :]
    return out
